# revision 15
# baseline (speedup 1.0000x reference)
"""Trainium2 Bass kernel for nn_ComplexAttention (B=8, C=512, H=W=32, HEADS=8).

Strategy
--------
Data-parallel over batch: one batch element per NeuronCore (8 cores), no
collectives.  Host-side algebraic fusion shrinks the per-core work:

  reference:  Q = R_q Wq Z,  K = R_k Wk Z,  V = R_v Wv Z   (complex, [C,T])
              S = Re(Q^H K)/sqrt(dh),  causal softmax -> A
              out = R_o Wo (V A^T)

  fused:      M = Wq^T diag(e^{i(phi_k-phi_q)}) Wk / sqrt(dh)   (host, f64)
              N = diag(e^{i phi_o}) Wo diag(e^{i phi_v}) Wv     (host, f64)
              Y = M Z            (channel-major [C,T])
              S = Re(Z^H Y)      = Zre^T Yre + Zim^T Yim
              A = softmax(causal(S))        (no max-subtraction: |S| < ~30)
              U = N Z            (token-major [T,C])
              out = U^T A^T      (channel-major [C,T], = re/im pair)

Per-core tensor-engine work is ~320 [128x128x512] matmuls + 36 transposes.
Matmuls run as float32r (full-rate fp32 PE mode; PSUM accumulates fp32).

Schedule notes (from HW traces):
 - input DMA is BW-bound (~358 GB/s/core), so loads are interleaved with
   the first matmul phases (mtre+zre -> Y_re, ntre -> U_re, zim -> rest).
 - softmax exp reads scores straight out of PSUM (no copy), per-chunk
   partial row-sums are added on DVE afterwards.
 - the softmax 1/l is folded into the PE transpose: P^T blocks are
   computed as block.T @ diag(1/l) (diag built via ident * rl on DVE).
 - t-tiles 4..7 are processed first so the final out chunk (t 512..1023)
   overlaps the scores/softmax of t-tiles 0..3.
"""

import math

import numpy as np

import concourse.mybir as mybir
import concourse.tile as tile
from concourse import bacc
from concourse.bass_utils import run_bass_kernel_spmd

B, C, HH, WW = 8, 512, 32, 32
T = HH * WW          # 1024 tokens
DH = C // 8          # head dim (scale only)
P = 128
CT = C // P          # 4 channel tiles
TT = T // P          # 8 token tiles
NEG = -1.0e30
DIAG_SCALE = False   # PE transpose mode requires a permutation matrix

f32 = mybir.dt.float32
f32r = mybir.dt.float32r


def _mm(nc, out, lhsT, rhs, start, stop):
    """matmul on float32r operands (1 cyc/row at N>=256)."""
    nc.tensor.matmul(out, lhsT, rhs, start=start, stop=stop)


_CACHE: dict = {}


def _get_program(has_imag: bool):
    key = has_imag
    if key not in _CACHE:
        _CACHE[key] = _build_program(has_imag)
    return _CACHE[key]


def _build_program(has_imag: bool):
    nc = bacc.Bacc("TRN2", target_bir_lowering=False, debug=False)

    zre_d = nc.dram_tensor("zre", [C, T], f32r, kind="ExternalInput").ap()
    zim_d = nc.dram_tensor("zim", [C, T], f32r, kind="ExternalInput").ap()
    mtre_d = nc.dram_tensor("mtre", [C, C], f32r, kind="ExternalInput").ap()
    ntre_d = nc.dram_tensor("ntre", [C, C], f32r, kind="ExternalInput").ap()
    if has_imag:
        mtim_d = nc.dram_tensor("mtim", [C, C], f32r, kind="ExternalInput").ap()
        mtimn_d = nc.dram_tensor("mtimn", [C, C], f32r, kind="ExternalInput").ap()
        ntim_d = nc.dram_tensor("ntim", [C, C], f32r, kind="ExternalInput").ap()
        ntimn_d = nc.dram_tensor("ntimn", [C, C], f32r, kind="ExternalInput").ap()
    ident_d = nc.dram_tensor("ident", [P, P], f32r, kind="ExternalInput").ap()
    tri_d = nc.dram_tensor("tri", [P, P], f32, kind="ExternalInput").ap()
    zpad_d = nc.dram_tensor("zpad", [P, 384], f32r, kind="ExternalInput").ap()
    outre_d = nc.dram_tensor("outre", [C, T], f32, kind="ExternalOutput").ap()
    outim_d = nc.dram_tensor("outim", [C, T], f32, kind="ExternalOutput").ap()

    with tile.TileContext(nc) as tc:
        with (
            tc.tile_pool(name="const", bufs=1) as cp,
            tc.tile_pool(name="work", bufs=3) as wp,
            tc.tile_pool(name="small", bufs=8) as sp,
            tc.tile_pool(name="psmm", bufs=4, space="PSUM") as pmm,
            tc.tile_pool(name="pstr", bufs=4, space="PSUM") as ptr,
        ):
            def load_rows_on(dram, tag, eng):
                tiles = []
                for c in range(CT):
                    t = cp.tile([P, C], f32r, tag=f"{tag}{c}",
                                name=f"{tag}{c}")
                    eng.dma_start(out=t, in_=dram[c * P:(c + 1) * P, :])
                    tiles.append(t)
                return tiles

            def load_half(dram, tag, half, eng):
                tiles = []
                for c in range(CT):
                    t = cp.tile([P, 512], f32r, tag=f"{tag}{c}_{half}",
                                name=f"{tag}{c}_{half}")
                    eng.dma_start(
                        out=t,
                        in_=dram[c * P:(c + 1) * P,
                                 half * 512:(half + 1) * 512])
                    tiles.append(t)
                return tiles

            # -- small constants + first compute inputs ---------------------
            # single sync queue (HBM BW is shared; parallel queues starve
            # the critical first loads), ordered by first use, with mtre/zre
            # interleaved per c-tile so accumulation starts after ~0.5MB.
            ident = cp.tile([P, P], f32r, tag="ident", name="ident")
            nc.sync.dma_start(out=ident, in_=ident_d)
            tri = cp.tile([P, P], f32, tag="tri", name="tri")
            nc.sync.dma_start(out=tri, in_=tri_d)
            mtre = [cp.tile([P, C], f32r, tag=f"mtre{c}", name=f"mtre{c}")
                    for c in range(CT)]
            zre_h = [[cp.tile([P, 512], f32r, tag=f"zre{c}_{h}",
                              name=f"zre{c}_{h}") for c in range(CT)]
                     for h in range(2)]
            for c in range(CT):
                nc.sync.dma_start(out=mtre[c],
                                  in_=mtre_d[c * P:(c + 1) * P, :])
                nc.sync.dma_start(out=zre_h[0][c],
                                  in_=zre_d[c * P:(c + 1) * P, 0:512])
            for c in range(CT):
                nc.sync.dma_start(out=zre_h[1][c],
                                  in_=zre_d[c * P:(c + 1) * P, 512:T])

            # persistent result tiles (split by column half: precise deps)
            yre = [[cp.tile([P, 512], f32r, tag=f"yre{c}_{n}",
                            name=f"yre{c}_{n}") for n in range(2)]
                   for c in range(CT)]
            yim = [[cp.tile([P, 512], f32r, tag=f"yim{c}_{n}",
                            name=f"yim{c}_{n}") for n in range(2)]
                   for c in range(CT)]
            ure = [cp.tile([P, C], f32r, tag=f"ure{j}", name=f"ure{j}")
                   for j in range(TT)]
            uim = [cp.tile([P, C], f32r, tag=f"uim{j}", name=f"uim{j}")
                   for j in range(TT)]

            def psum_to_sbuf(dst_ap, src_ap):
                nc.vector.tensor_copy(out=dst_ap, in_=src_ap)

            def emit_y(dst, terms):
                nterm = len(terms)
                for n in range(2):
                    pss = [pmm.tile([P, 512], f32, tag="mm", name="psmm")
                           for _ in range(CT)]
                    for t_i, (w, zh) in enumerate(terms):
                        for c in range(CT):
                            for m in range(CT):
                                _mm(nc, pss[m], w[c][:, m * P:(m + 1) * P],
                                    zh[n][c],
                                    start=(t_i == 0 and c == 0),
                                    stop=(t_i == nterm - 1 and c == CT - 1))
                    for m in range(CT):
                        psum_to_sbuf(dst[m][n], pss[m])

            def emit_u(dst, terms):
                for j in range(TT):
                    usl = slice((j % 4) * P, (j % 4 + 1) * P)
                    ps = pmm.tile([P, 512], f32, tag="mm", name="psmm")
                    nacc = len(terms) * CT
                    k = 0
                    for zh, w in terms:
                        for c in range(CT):
                            _mm(nc, ps, zh[j // 4][c][:, usl], w[c][:, :],
                                start=(k == 0), stop=(k == nacc - 1))
                            k += 1
                    psum_to_sbuf(dst[j], ps)

            # -- Y_re (needs mtre+zre only), then stream in the rest.
            # Later loads are EMITTED after emit_y so the watermark-style
            # sem waits on the first matmuls don't cover them; the DMA
            # engines still run their own streams immediately.
            if not has_imag:
                emit_y(yre, [(mtre, zre_h)])
                ntre = load_rows_on(ntre_d, "ntre", nc.sync)
                zim_h = [load_half(zim_d, "zim", 0, nc.sync),
                         load_half(zim_d, "zim", 1, nc.sync)]
                emit_u(ure, [(zre_h, ntre)])
                emit_y(yim, [(mtre, zim_h)])
                emit_u(uim, [(zim_h, ntre)])
            else:
                zim_h = [load_half(zim_d, "zim", 0, nc.sync),
                         load_half(zim_d, "zim", 1, nc.sync)]
                mtim = load_rows_on(mtim_d, "mtim", nc.sync)
                mtimn = load_rows_on(mtimn_d, "mtimn", nc.sync)
                ntre = load_rows_on(ntre_d, "ntre", nc.sync)
                ntim = load_rows_on(ntim_d, "ntim", nc.sync)
                ntimn = load_rows_on(ntimn_d, "ntimn", nc.sync)
                emit_y(yre, [(mtre, zre_h), (mtimn, zim_h)])
                emit_y(yim, [(mtre, zim_h), (mtim, zre_h)])
                emit_u(ure, [(zre_h, ntre), (zim_h, ntimn)])
                emit_u(uim, [(zim_h, ntre), (zre_h, ntim)])

            # -- P^T blocks (u-tile j, t-chunk n); zero upper regions -------
            pt = {}
            for j in range(TT):
                for n in range(2):
                    if n == 0 and j >= 4:
                        continue
                    ptile = cp.tile([P, 512], f32r, tag=f"pt{j}_{n}",
                                    name=f"pt{j}_{n}")
                    pt[(j, n)] = ptile

            def emit_out_chunk(n, half=None, cols=(0, 512)):
                """out[:, n*512+cols] = U^T @ P^T for re and/or im."""
                c0, c1 = cols
                width = c1 - c0
                jmax = 4 * n + 3
                tsl = slice(n * 512 + c0, n * 512 + c1)
                pairs = ((ure, outre_d, nc.sync), (uim, outim_d, nc.sync))
                if half is not None:
                    pairs = (pairs[half],)
                for u, dram, oeng in pairs:
                    for m in range(CT):
                        msl = slice(m * P, (m + 1) * P)
                        ps = pmm.tile([P, 512], f32, tag="mm", name="psmm")
                        started = False
                        for j in range(jmax + 1):
                            # pt[(j, n)] is all-zero left of column lo
                            lo = max(c0, j * P - n * 512)
                            if lo >= c1:
                                continue
                            _mm(nc, ps[:, lo - c0: width],
                                u[j][:, msl], pt[(j, n)][:, lo:c1],
                                start=(not started), stop=(j == jmax))
                            started = True
                        o = wp.tile([P, 512], f32, tag="osb", name="osb")
                        psum_to_sbuf(o[:, :width], ps[:, :width])
                        oeng.dma_start(out=dram[msl, tsl], in_=o[:, :width])

            # -- scores / softmax / transposes per t-tile -------------------
            def emit_scores_tile(i):
                ui = (i + 1) * P
                isl = slice((i % 4) * P, (i % 4 + 1) * P)
                s_sb = wp.tile([P, T], f32r, tag="s", name="s_sb")
                nchunks = (ui + 511) // 512
                lparts = []
                for q in range(nchunks):
                    w = min(512, ui - q * 512)
                    ps = pmm.tile([P, 512], f32, tag="mm", name="psmm")
                    k = 0
                    for zh, y in ((zre_h, yre), (zim_h, yim)):
                        for c in range(CT):
                            _mm(nc, ps[:, :w], zh[i // 4][c][:, isl],
                                y[c][q][:, :w],
                                start=(k == 0), stop=(k == 2 * CT - 1))
                            k += 1
                    last = q == nchunks - 1
                    if last:
                        if w > P:
                            # non-frontier part: exp straight from PSUM
                            lp = sp.tile([P, 1], f32, tag="lp", name="lp")
                            nc.scalar.activation(
                                out=s_sb[:, q * 512: q * 512 + w - P],
                                in_=ps[:, : w - P],
                                func=mybir.ActivationFunctionType.Exp,
                                accum_out=lp,
                            )
                            lparts.append(lp)
                        # frontier 128 cols: +tri mask (DVE), then exp
                        fr = sp.tile([P, P], f32, tag="fr", name="fr")
                        nc.vector.tensor_add(out=fr, in0=ps[:, w - P: w],
                                             in1=tri)
                        lp = sp.tile([P, 1], f32, tag="lp", name="lp")
                        nc.scalar.activation(
                            out=s_sb[:, ui - P: ui], in_=fr,
                            func=mybir.ActivationFunctionType.Exp,
                            accum_out=lp,
                        )
                        lparts.append(lp)
                    else:
                        lp = sp.tile([P, 1], f32, tag="lp", name="lp")
                        nc.scalar.activation(
                            out=s_sb[:, q * 512: q * 512 + w],
                            in_=ps[:, :w],
                            func=mybir.ActivationFunctionType.Exp,
                            accum_out=lp,
                        )
                        lparts.append(lp)

                lsum = lparts[0]
                for extra in lparts[1:]:
                    acc = sp.tile([P, 1], f32, tag="lacc", name="lacc")
                    nc.vector.tensor_add(out=acc, in0=lsum, in1=extra)
                    lsum = acc
                rl = sp.tile([P, 1], f32, tag="rl", name="rl")
                nc.vector.reciprocal(out=rl, in_=lsum)

                if DIAG_SCALE:
                    dg = sp.tile([P, P], f32r, tag="dg", name="dg")
                    nc.vector.tensor_scalar_mul(dg, ident, rl)
                    rhs = dg
                else:
                    nc.vector.tensor_scalar_mul(s_sb[:, :ui], s_sb[:, :ui],
                                                rl)
                    rhs = ident

                n = i // 4
                for j in range(i + 1):
                    pstile = ptr.tile([P, P], f32r, tag="tr", name="pstile")
                    nc.tensor.transpose(pstile, s_sb[:, j * P:(j + 1) * P],
                                        rhs)
                    nc.vector.tensor_copy(
                        out=pt[(j, n)][:, i * P - n * 512:
                                       (i + 1) * P - n * 512],
                        in_=pstile,
                    )

            for i in (4, 5, 6, 3):
                emit_scores_tile(i)
            emit_scores_tile(7)
            emit_scores_tile(2)
            emit_out_chunk(1, half=0)
            emit_scores_tile(1)
            emit_out_chunk(1, half=1)
            emit_out_chunk(0, half=0, cols=(256, 512))
            emit_out_chunk(0, half=1, cols=(256, 512))
            emit_scores_tile(0)
            emit_out_chunk(0, half=0, cols=(0, 256))
            emit_out_chunk(0, half=1, cols=(0, 256))

    nc.compile()
    return nc


def _prep_weights(Wq, phi_q, Wk, phi_k, Wv, phi_v, Wo, phi_o):
    Wq, Wk, Wv, Wo = (np.asarray(w, np.float64) for w in (Wq, Wk, Wv, Wo))
    pq, pk, pv, po = (np.asarray(p, np.float64)
                      for p in (phi_q, phi_k, phi_v, phi_o))
    M = (Wq.T @ (np.exp(1j * (pk - pq))[:, None] * Wk)) / math.sqrt(DH)
    N = (np.exp(1j * po)[:, None] * Wo) @ (np.exp(1j * pv)[:, None] * Wv)
    has_imag = not (np.allclose(M.imag, 0.0) and np.allclose(N.imag, 0.0))
    return M, N, has_imag


def kernel(z_re, z_im, Wq, phi_q, Wk, phi_k, Wv, phi_v, Wo, phi_o):
    z_re = np.ascontiguousarray(np.asarray(z_re, np.float32))
    z_im = np.ascontiguousarray(np.asarray(z_im, np.float32))
    M, N, has_imag = _prep_weights(Wq, phi_q, Wk, phi_k, Wv, phi_v, Wo, phi_o)

    mtre = np.ascontiguousarray(M.real.T.astype(np.float32))
    ntre = np.ascontiguousarray(N.real.T.astype(np.float32))
    consts = {"mtre": mtre, "ntre": ntre}
    if has_imag:
        mtim = np.ascontiguousarray(M.imag.T.astype(np.float32))
        ntim = np.ascontiguousarray(N.imag.T.astype(np.float32))
        consts.update(mtim=mtim, mtimn=-mtim, ntim=ntim, ntimn=-ntim)

    consts["ident"] = np.eye(P, dtype=np.float32)
    consts["tri"] = np.triu(np.full((P, P), NEG, np.float32), 1)
    consts["zpad"] = np.zeros((P, 384), np.float32)
    nc = _get_program(has_imag)
    in_maps = [
        dict(consts, zre=z_re[b].reshape(C, T), zim=z_im[b].reshape(C, T))
        for b in range(B)
    ]
    res = run_bass_kernel_spmd(nc, in_maps, list(range(B)))
    out_re = np.stack([res.results[b]["outre"].reshape(C, HH, WW)
                       for b in range(B)])
    out_im = np.stack([res.results[b]["outim"].reshape(C, HH, WW)
                       for b in range(B)])
    return out_re, out_im


# revision 16
# speedup vs baseline: 1.0031x; 1.0031x over previous
"""Trainium2 Bass kernel for nn_ComplexAttention (B=8, C=512, H=W=32, HEADS=8).

Strategy
--------
Data-parallel over batch: one batch element per NeuronCore (8 cores), no
collectives.  Host-side algebraic fusion shrinks the per-core work:

  reference:  Q = R_q Wq Z,  K = R_k Wk Z,  V = R_v Wv Z   (complex, [C,T])
              S = Re(Q^H K)/sqrt(dh),  causal softmax -> A
              out = R_o Wo (V A^T)

  fused:      M = Wq^T diag(e^{i(phi_k-phi_q)}) Wk / sqrt(dh)   (host, f64)
              N = diag(e^{i phi_o}) Wo diag(e^{i phi_v}) Wv     (host, f64)
              Y = M Z            (channel-major [C,T])
              S = Re(Z^H Y)      = Zre^T Yre + Zim^T Yim
              A = softmax(causal(S))        (no max-subtraction: |S| < ~30)
              U = N Z            (token-major [T,C])
              out = U^T A^T      (channel-major [C,T], = re/im pair)

Per-core tensor-engine work is ~320 [128x128x512] matmuls + 36 transposes.
Matmuls run as float32r (full-rate fp32 PE mode; PSUM accumulates fp32).

Schedule notes (from HW traces):
 - input DMA is BW-bound (~358 GB/s/core), so loads are interleaved with
   the first matmul phases (mtre+zre -> Y_re, ntre -> U_re, zim -> rest).
 - softmax exp reads scores straight out of PSUM (no copy), per-chunk
   partial row-sums are added on DVE afterwards.
 - the softmax 1/l is folded into the PE transpose: P^T blocks are
   computed as block.T @ diag(1/l) (diag built via ident * rl on DVE).
 - t-tiles 4..7 are processed first so the final out chunk (t 512..1023)
   overlaps the scores/softmax of t-tiles 0..3.
"""

import math

import numpy as np

import concourse.mybir as mybir
import concourse.tile as tile
from concourse import bacc
from concourse.bass_utils import run_bass_kernel_spmd

B, C, HH, WW = 8, 512, 32, 32
T = HH * WW          # 1024 tokens
DH = C // 8          # head dim (scale only)
P = 128
CT = C // P          # 4 channel tiles
TT = T // P          # 8 token tiles
NEG = -1.0e30
DIAG_SCALE = False   # PE transpose mode requires a permutation matrix

f32 = mybir.dt.float32
f32r = mybir.dt.float32r


def _mm(nc, out, lhsT, rhs, start, stop):
    """matmul on float32r operands (1 cyc/row at N>=256)."""
    nc.tensor.matmul(out, lhsT, rhs, start=start, stop=stop)


_CACHE: dict = {}


def _get_program(has_imag: bool):
    key = has_imag
    if key not in _CACHE:
        _CACHE[key] = _build_program(has_imag)
    return _CACHE[key]


def _build_program(has_imag: bool):
    nc = bacc.Bacc("TRN2", target_bir_lowering=False, debug=False)

    zre_d = nc.dram_tensor("zre", [C, T], f32r, kind="ExternalInput").ap()
    zim_d = nc.dram_tensor("zim", [C, T], f32r, kind="ExternalInput").ap()
    mtre_d = nc.dram_tensor("mtre", [C, C], f32r, kind="ExternalInput").ap()
    ntre_d = nc.dram_tensor("ntre", [C, C], f32r, kind="ExternalInput").ap()
    if has_imag:
        mtim_d = nc.dram_tensor("mtim", [C, C], f32r, kind="ExternalInput").ap()
        mtimn_d = nc.dram_tensor("mtimn", [C, C], f32r, kind="ExternalInput").ap()
        ntim_d = nc.dram_tensor("ntim", [C, C], f32r, kind="ExternalInput").ap()
        ntimn_d = nc.dram_tensor("ntimn", [C, C], f32r, kind="ExternalInput").ap()
    ident_d = nc.dram_tensor("ident", [P, P], f32r, kind="ExternalInput").ap()
    tri_d = nc.dram_tensor("tri", [P, P], f32, kind="ExternalInput").ap()
    zpad_d = nc.dram_tensor("zpad", [P, 384], f32r, kind="ExternalInput").ap()
    outre_d = nc.dram_tensor("outre", [C, T], f32, kind="ExternalOutput").ap()
    outim_d = nc.dram_tensor("outim", [C, T], f32, kind="ExternalOutput").ap()

    with tile.TileContext(nc) as tc:
        with (
            tc.tile_pool(name="const", bufs=1) as cp,
            tc.tile_pool(name="work", bufs=3) as wp,
            tc.tile_pool(name="small", bufs=8) as sp,
            tc.tile_pool(name="psmm", bufs=4, space="PSUM") as pmm,
            tc.tile_pool(name="pstr", bufs=4, space="PSUM") as ptr,
        ):
            def load_rows_on(dram, tag, eng):
                tiles = []
                for c in range(CT):
                    t = cp.tile([P, C], f32r, tag=f"{tag}{c}",
                                name=f"{tag}{c}")
                    eng.dma_start(out=t, in_=dram[c * P:(c + 1) * P, :])
                    tiles.append(t)
                return tiles

            def load_half(dram, tag, half, eng):
                tiles = []
                for c in range(CT):
                    t = cp.tile([P, 512], f32r, tag=f"{tag}{c}_{half}",
                                name=f"{tag}{c}_{half}")
                    eng.dma_start(
                        out=t,
                        in_=dram[c * P:(c + 1) * P,
                                 half * 512:(half + 1) * 512])
                    tiles.append(t)
                return tiles

            # -- small constants + first compute inputs ---------------------
            # single sync queue (HBM BW is shared; parallel queues starve
            # the critical first loads), ordered by first use, with mtre/zre
            # interleaved per c-tile so accumulation starts after ~0.5MB.
            ident = cp.tile([P, P], f32r, tag="ident", name="ident")
            nc.sync.dma_start(out=ident, in_=ident_d)
            tri = cp.tile([P, P], f32, tag="tri", name="tri")
            nc.sync.dma_start(out=tri, in_=tri_d)
            mtre = [cp.tile([P, C], f32r, tag=f"mtre{c}", name=f"mtre{c}")
                    for c in range(CT)]
            zre_h = [[cp.tile([P, 512], f32r, tag=f"zre{c}_{h}",
                              name=f"zre{c}_{h}") for c in range(CT)]
                     for h in range(2)]
            for c in range(CT):
                nc.sync.dma_start(out=mtre[c],
                                  in_=mtre_d[c * P:(c + 1) * P, :])
                nc.sync.dma_start(out=zre_h[0][c],
                                  in_=zre_d[c * P:(c + 1) * P, 0:512])
            for c in range(CT):
                nc.sync.dma_start(out=zre_h[1][c],
                                  in_=zre_d[c * P:(c + 1) * P, 512:T])

            # persistent result tiles (split by column half: precise deps)
            yre = [[cp.tile([P, 512], f32r, tag=f"yre{c}_{n}",
                            name=f"yre{c}_{n}") for n in range(2)]
                   for c in range(CT)]
            yim = [[cp.tile([P, 512], f32r, tag=f"yim{c}_{n}",
                            name=f"yim{c}_{n}") for n in range(2)]
                   for c in range(CT)]
            ure = [cp.tile([P, C], f32r, tag=f"ure{j}", name=f"ure{j}")
                   for j in range(TT)]
            uim = [cp.tile([P, C], f32r, tag=f"uim{j}", name=f"uim{j}")
                   for j in range(TT)]

            def psum_to_sbuf(dst_ap, src_ap):
                nc.vector.tensor_copy(out=dst_ap, in_=src_ap)

            def emit_y(dst, terms):
                nterm = len(terms)
                for n in range(2):
                    pss = [pmm.tile([P, 512], f32, tag="mm", name="psmm")
                           for _ in range(CT)]
                    for t_i, (w, zh) in enumerate(terms):
                        for c in range(CT):
                            for m in range(CT):
                                _mm(nc, pss[m], w[c][:, m * P:(m + 1) * P],
                                    zh[n][c],
                                    start=(t_i == 0 and c == 0),
                                    stop=(t_i == nterm - 1 and c == CT - 1))
                    for m in range(CT):
                        psum_to_sbuf(dst[m][n], pss[m])

            def emit_u(dst, terms):
                for j in range(TT):
                    usl = slice((j % 4) * P, (j % 4 + 1) * P)
                    ps = pmm.tile([P, 512], f32, tag="mm", name="psmm")
                    nacc = len(terms) * CT
                    k = 0
                    for zh, w in terms:
                        for c in range(CT):
                            _mm(nc, ps, zh[j // 4][c][:, usl], w[c][:, :],
                                start=(k == 0), stop=(k == nacc - 1))
                            k += 1
                    psum_to_sbuf(dst[j], ps)

            # -- Y_re (needs mtre+zre only), then stream in the rest.
            # Later loads are EMITTED after emit_y so the watermark-style
            # sem waits on the first matmuls don't cover them; the DMA
            # engines still run their own streams immediately.
            if not has_imag:
                emit_y(yre, [(mtre, zre_h)])
                ntre = load_rows_on(ntre_d, "ntre", nc.sync)
                zim_h = [load_half(zim_d, "zim", 0, nc.sync),
                         load_half(zim_d, "zim", 1, nc.sync)]
                emit_u(ure, [(zre_h, ntre)])
                emit_y(yim, [(mtre, zim_h)])
                emit_u(uim, [(zim_h, ntre)])
            else:
                zim_h = [load_half(zim_d, "zim", 0, nc.sync),
                         load_half(zim_d, "zim", 1, nc.sync)]
                mtim = load_rows_on(mtim_d, "mtim", nc.sync)
                mtimn = load_rows_on(mtimn_d, "mtimn", nc.sync)
                ntre = load_rows_on(ntre_d, "ntre", nc.sync)
                ntim = load_rows_on(ntim_d, "ntim", nc.sync)
                ntimn = load_rows_on(ntimn_d, "ntimn", nc.sync)
                emit_y(yre, [(mtre, zre_h), (mtimn, zim_h)])
                emit_y(yim, [(mtre, zim_h), (mtim, zre_h)])
                emit_u(ure, [(zre_h, ntre), (zim_h, ntimn)])
                emit_u(uim, [(zim_h, ntre), (zre_h, ntim)])

            # -- P^T blocks (u-tile j, t-chunk n); zero upper regions -------
            pt = {}
            for j in range(TT):
                for n in range(2):
                    if n == 0 and j >= 4:
                        continue
                    ptile = cp.tile([P, 512], f32r, tag=f"pt{j}_{n}",
                                    name=f"pt{j}_{n}")
                    pt[(j, n)] = ptile

            def emit_out_chunk(n, half=None, cols=(0, 512)):
                """out[:, n*512+cols] = U^T @ P^T for re and/or im."""
                c0, c1 = cols
                width = c1 - c0
                jmax = 4 * n + 3
                tsl = slice(n * 512 + c0, n * 512 + c1)
                pairs = ((ure, outre_d, nc.sync), (uim, outim_d, nc.sync))
                if half is not None:
                    pairs = (pairs[half],)
                js = [j for j in range(jmax + 1)
                      if max(c0, j * P - n * 512) < c1]
                for u, dram, oeng in pairs:
                    for m in range(CT):
                        msl = slice(m * P, (m + 1) * P)
                        ps = pmm.tile([P, 512], f32, tag="mm", name="psmm")
                        for j in js:
                            # pt[(j, n)] is all-zero left of column lo
                            lo = max(c0, j * P - n * 512)
                            _mm(nc, ps[:, lo - c0: width],
                                u[j][:, msl], pt[(j, n)][:, lo:c1],
                                start=(j == js[0]), stop=(j == js[-1]))
                        o = wp.tile([P, 512], f32, tag="osb", name="osb")
                        psum_to_sbuf(o[:, :width], ps[:, :width])
                        oeng.dma_start(out=dram[msl, tsl], in_=o[:, :width])

            # -- scores / softmax / transposes per t-tile -------------------
            def emit_scores_tile(i):
                ui = (i + 1) * P
                isl = slice((i % 4) * P, (i % 4 + 1) * P)
                s_sb = wp.tile([P, T], f32r, tag="s", name="s_sb")
                nchunks = (ui + 511) // 512
                lparts = []
                for q in range(nchunks):
                    w = min(512, ui - q * 512)
                    ps = pmm.tile([P, 512], f32, tag="mm", name="psmm")
                    k = 0
                    for zh, y in ((zre_h, yre), (zim_h, yim)):
                        for c in range(CT):
                            _mm(nc, ps[:, :w], zh[i // 4][c][:, isl],
                                y[c][q][:, :w],
                                start=(k == 0), stop=(k == 2 * CT - 1))
                            k += 1
                    last = q == nchunks - 1
                    if last:
                        if w > P:
                            # non-frontier part: exp straight from PSUM
                            lp = sp.tile([P, 1], f32, tag="lp", name="lp")
                            nc.scalar.activation(
                                out=s_sb[:, q * 512: q * 512 + w - P],
                                in_=ps[:, : w - P],
                                func=mybir.ActivationFunctionType.Exp,
                                accum_out=lp,
                            )
                            lparts.append(lp)
                        # frontier 128 cols: +tri mask (DVE), then exp
                        fr = sp.tile([P, P], f32, tag="fr", name="fr")
                        nc.vector.tensor_add(out=fr, in0=ps[:, w - P: w],
                                             in1=tri)
                        lp = sp.tile([P, 1], f32, tag="lp", name="lp")
                        nc.scalar.activation(
                            out=s_sb[:, ui - P: ui], in_=fr,
                            func=mybir.ActivationFunctionType.Exp,
                            accum_out=lp,
                        )
                        lparts.append(lp)
                    else:
                        lp = sp.tile([P, 1], f32, tag="lp", name="lp")
                        nc.scalar.activation(
                            out=s_sb[:, q * 512: q * 512 + w],
                            in_=ps[:, :w],
                            func=mybir.ActivationFunctionType.Exp,
                            accum_out=lp,
                        )
                        lparts.append(lp)

                lsum = lparts[0]
                for extra in lparts[1:]:
                    acc = sp.tile([P, 1], f32, tag="lacc", name="lacc")
                    nc.vector.tensor_add(out=acc, in0=lsum, in1=extra)
                    lsum = acc
                rl = sp.tile([P, 1], f32, tag="rl", name="rl")
                nc.vector.reciprocal(out=rl, in_=lsum)

                if DIAG_SCALE:
                    dg = sp.tile([P, P], f32r, tag="dg", name="dg")
                    nc.vector.tensor_scalar_mul(dg, ident, rl)
                    rhs = dg
                else:
                    nc.vector.tensor_scalar_mul(s_sb[:, :ui], s_sb[:, :ui],
                                                rl)
                    rhs = ident

                n = i // 4
                for j in range(i + 1):
                    pstile = ptr.tile([P, P], f32r, tag="tr", name="pstile")
                    nc.tensor.transpose(pstile, s_sb[:, j * P:(j + 1) * P],
                                        rhs)
                    nc.vector.tensor_copy(
                        out=pt[(j, n)][:, i * P - n * 512:
                                       (i + 1) * P - n * 512],
                        in_=pstile,
                    )

            for i in (4, 5, 6, 3):
                emit_scores_tile(i)
            emit_scores_tile(7)
            emit_scores_tile(2)
            emit_out_chunk(1, half=0)
            emit_scores_tile(1)
            emit_out_chunk(1, half=1)
            emit_out_chunk(0, half=0, cols=(256, 512))
            emit_out_chunk(0, half=1, cols=(256, 512))
            emit_scores_tile(0)
            emit_out_chunk(0, half=0, cols=(0, 256))
            emit_out_chunk(0, half=1, cols=(0, 256))

    nc.compile()
    return nc


def _prep_weights(Wq, phi_q, Wk, phi_k, Wv, phi_v, Wo, phi_o):
    Wq, Wk, Wv, Wo = (np.asarray(w, np.float64) for w in (Wq, Wk, Wv, Wo))
    pq, pk, pv, po = (np.asarray(p, np.float64)
                      for p in (phi_q, phi_k, phi_v, phi_o))
    M = (Wq.T @ (np.exp(1j * (pk - pq))[:, None] * Wk)) / math.sqrt(DH)
    N = (np.exp(1j * po)[:, None] * Wo) @ (np.exp(1j * pv)[:, None] * Wv)
    has_imag = not (np.allclose(M.imag, 0.0) and np.allclose(N.imag, 0.0))
    return M, N, has_imag


def kernel(z_re, z_im, Wq, phi_q, Wk, phi_k, Wv, phi_v, Wo, phi_o):
    z_re = np.ascontiguousarray(np.asarray(z_re, np.float32))
    z_im = np.ascontiguousarray(np.asarray(z_im, np.float32))
    M, N, has_imag = _prep_weights(Wq, phi_q, Wk, phi_k, Wv, phi_v, Wo, phi_o)

    mtre = np.ascontiguousarray(M.real.T.astype(np.float32))
    ntre = np.ascontiguousarray(N.real.T.astype(np.float32))
    consts = {"mtre": mtre, "ntre": ntre}
    if has_imag:
        mtim = np.ascontiguousarray(M.imag.T.astype(np.float32))
        ntim = np.ascontiguousarray(N.imag.T.astype(np.float32))
        consts.update(mtim=mtim, mtimn=-mtim, ntim=ntim, ntimn=-ntim)

    consts["ident"] = np.eye(P, dtype=np.float32)
    consts["tri"] = np.triu(np.full((P, P), NEG, np.float32), 1)
    consts["zpad"] = np.zeros((P, 384), np.float32)
    nc = _get_program(has_imag)
    in_maps = [
        dict(consts, zre=z_re[b].reshape(C, T), zim=z_im[b].reshape(C, T))
        for b in range(B)
    ]
    res = run_bass_kernel_spmd(nc, in_maps, list(range(B)))
    out_re = np.stack([res.results[b]["outre"].reshape(C, HH, WW)
                       for b in range(B)])
    out_im = np.stack([res.results[b]["outim"].reshape(C, HH, WW)
                       for b in range(B)])
    return out_re, out_im


# revision 19
# speedup vs baseline: 1.0639x; 1.0605x over previous
"""Trainium2 Bass kernel for nn_ComplexAttention (B=8, C=512, H=W=32, HEADS=8).

Strategy
--------
Data-parallel over batch: one batch element per NeuronCore (8 cores), no
collectives.  Host-side algebraic fusion shrinks the per-core work:

  reference:  Q = R_q Wq Z,  K = R_k Wk Z,  V = R_v Wv Z   (complex, [C,T])
              S = Re(Q^H K)/sqrt(dh),  causal softmax -> A
              out = R_o Wo (V A^T)

  fused:      M = Wq^T diag(e^{i(phi_k-phi_q)}) Wk / sqrt(dh)   (host, f64)
              N = diag(e^{i phi_o}) Wo diag(e^{i phi_v}) Wv     (host, f64)
              Y = M Z            (channel-major [C,T])
              S = Re(Z^H Y)      = Zre^T Yre + Zim^T Yim
              A = softmax(causal(S))        (no max-subtraction: |S| < ~30)
              U = N Z            (token-major [T,C])
              out = U^T A^T      (channel-major [C,T], = re/im pair)

Per-core tensor-engine work is ~320 [128x128x512] matmuls + 36 transposes.
Matmuls run as float32r (full-rate fp32 PE mode; PSUM accumulates fp32).

Schedule notes (from HW traces):
 - input DMA is BW-bound (~358 GB/s/core), so loads are interleaved with
   the first matmul phases (mtre+zre -> Y_re, ntre -> U_re, zim -> rest).
 - softmax exp reads scores straight out of PSUM (no copy), per-chunk
   partial row-sums are added on DVE afterwards.
 - the softmax 1/l is folded into the PE transpose: P^T blocks are
   computed as block.T @ diag(1/l) (diag built via ident * rl on DVE).
 - t-tiles 4..7 are processed first so the final out chunk (t 512..1023)
   overlaps the scores/softmax of t-tiles 0..3.
"""

import math

import numpy as np

import concourse.mybir as mybir
import concourse.tile as tile
from concourse import bacc
from concourse.bass_utils import run_bass_kernel_spmd

B, C, HH, WW = 8, 512, 32, 32
T = HH * WW          # 1024 tokens
DH = C // 8          # head dim (scale only)
P = 128
CT = C // P          # 4 channel tiles
TT = T // P          # 8 token tiles
NEG = -1.0e30
DIAG_SCALE = False   # PE transpose mode requires a permutation matrix

f32 = mybir.dt.float32
f32r = mybir.dt.float32r


def _mm(nc, out, lhsT, rhs, start, stop):
    """matmul on float32r operands (1 cyc/row at N>=256)."""
    nc.tensor.matmul(out, lhsT, rhs, start=start, stop=stop)


_CACHE: dict = {}


def _get_program(has_imag: bool):
    key = has_imag
    if key not in _CACHE:
        _CACHE[key] = _build_program(has_imag)
    return _CACHE[key]


def _build_program(has_imag: bool):
    nc = bacc.Bacc("TRN2", target_bir_lowering=False, debug=False)

    zre_d = nc.dram_tensor("zre", [C, T], f32r, kind="ExternalInput").ap()
    zim_d = nc.dram_tensor("zim", [C, T], f32r, kind="ExternalInput").ap()
    mtre_d = nc.dram_tensor("mtre", [C, C], f32r, kind="ExternalInput").ap()
    ntre_d = nc.dram_tensor("ntre", [C, C], f32r, kind="ExternalInput").ap()
    if has_imag:
        mtim_d = nc.dram_tensor("mtim", [C, C], f32r, kind="ExternalInput").ap()
        mtimn_d = nc.dram_tensor("mtimn", [C, C], f32r, kind="ExternalInput").ap()
        ntim_d = nc.dram_tensor("ntim", [C, C], f32r, kind="ExternalInput").ap()
        ntimn_d = nc.dram_tensor("ntimn", [C, C], f32r, kind="ExternalInput").ap()
    ident_d = nc.dram_tensor("ident", [P, P], f32r, kind="ExternalInput").ap()
    tri_d = nc.dram_tensor("tri", [P, P], f32, kind="ExternalInput").ap()
    trif_d = nc.dram_tensor("trif", [P, 256], f32, kind="ExternalInput").ap()
    zpad_d = nc.dram_tensor("zpad", [P, 384], f32r, kind="ExternalInput").ap()
    outre_d = nc.dram_tensor("outre", [C, T], f32, kind="ExternalOutput").ap()
    outim_d = nc.dram_tensor("outim", [C, T], f32, kind="ExternalOutput").ap()

    with tile.TileContext(nc) as tc:
        with (
            tc.tile_pool(name="const", bufs=1) as cp,
            tc.tile_pool(name="work", bufs=3) as wp,
            tc.tile_pool(name="small", bufs=8) as sp,
            tc.tile_pool(name="psmm", bufs=4, space="PSUM") as pmm,
            tc.tile_pool(name="pstr", bufs=4, space="PSUM") as ptr,
        ):
            def load_rows_on(dram, tag, eng):
                tiles = []
                for c in range(CT):
                    t = cp.tile([P, C], f32r, tag=f"{tag}{c}",
                                name=f"{tag}{c}")
                    eng.dma_start(out=t, in_=dram[c * P:(c + 1) * P, :])
                    tiles.append(t)
                return tiles

            def load_half(dram, tag, half, eng):
                tiles = []
                for c in range(CT):
                    t = cp.tile([P, 512], f32r, tag=f"{tag}{c}_{half}",
                                name=f"{tag}{c}_{half}")
                    eng.dma_start(
                        out=t,
                        in_=dram[c * P:(c + 1) * P,
                                 half * 512:(half + 1) * 512])
                    tiles.append(t)
                return tiles

            # -- small constants + first compute inputs ---------------------
            # single sync queue (HBM BW is shared; parallel queues starve
            # the critical first loads), ordered by first use, with mtre/zre
            # interleaved per c-tile so accumulation starts after ~0.5MB.
            ident = cp.tile([P, P], f32r, tag="ident", name="ident")
            nc.sync.dma_start(out=ident, in_=ident_d)
            tri = cp.tile([P, P], f32, tag="tri", name="tri")
            nc.sync.dma_start(out=tri, in_=tri_d)
            trif = cp.tile([P, 256], f32, tag="trif", name="trif")
            nc.sync.dma_start(out=trif, in_=trif_d)
            mtre = [cp.tile([P, C], f32r, tag=f"mtre{c}", name=f"mtre{c}")
                    for c in range(CT)]
            zre_h = [[cp.tile([P, 512], f32r, tag=f"zre{c}_{h}",
                              name=f"zre{c}_{h}") for c in range(CT)]
                     for h in range(2)]
            for c in range(CT):
                nc.sync.dma_start(out=mtre[c],
                                  in_=mtre_d[c * P:(c + 1) * P, :])
                nc.sync.dma_start(out=zre_h[0][c],
                                  in_=zre_d[c * P:(c + 1) * P, 0:512])
            for c in range(CT):
                nc.sync.dma_start(out=zre_h[1][c],
                                  in_=zre_d[c * P:(c + 1) * P, 512:T])

            # persistent result tiles (split by column half: precise deps)
            yre = [[cp.tile([P, 512], f32r, tag=f"yre{c}_{n}",
                            name=f"yre{c}_{n}") for n in range(2)]
                   for c in range(CT)]
            yim = [[cp.tile([P, 512], f32r, tag=f"yim{c}_{n}",
                            name=f"yim{c}_{n}") for n in range(2)]
                   for c in range(CT)]
            ure = [cp.tile([P, C], f32r, tag=f"ure{j}", name=f"ure{j}")
                   for j in range(TT)]
            uim = [cp.tile([P, C], f32r, tag=f"uim{j}", name=f"uim{j}")
                   for j in range(TT)]

            def psum_to_sbuf(dst_ap, src_ap):
                nc.vector.tensor_copy(out=dst_ap, in_=src_ap)

            def emit_y(dst, terms):
                nterm = len(terms)
                for n in range(2):
                    pss = [pmm.tile([P, 512], f32, tag="mm", name="psmm")
                           for _ in range(CT)]
                    for t_i, (w, zh) in enumerate(terms):
                        for c in range(CT):
                            for m in range(CT):
                                _mm(nc, pss[m], w[c][:, m * P:(m + 1) * P],
                                    zh[n][c],
                                    start=(t_i == 0 and c == 0),
                                    stop=(t_i == nterm - 1 and c == CT - 1))
                    for m in range(CT):
                        psum_to_sbuf(dst[m][n], pss[m])

            def emit_u(dst, terms):
                for j in range(TT):
                    usl = slice((j % 4) * P, (j % 4 + 1) * P)
                    ps = pmm.tile([P, 512], f32, tag="mm", name="psmm")
                    nacc = len(terms) * CT
                    k = 0
                    for zh, w in terms:
                        for c in range(CT):
                            _mm(nc, ps, zh[j // 4][c][:, usl], w[c][:, :],
                                start=(k == 0), stop=(k == nacc - 1))
                            k += 1
                    psum_to_sbuf(dst[j], ps)

            # -- Y_re (needs mtre+zre only), then stream in the rest.
            # Later loads are EMITTED after emit_y so the watermark-style
            # sem waits on the first matmuls don't cover them; the DMA
            # engines still run their own streams immediately.
            if not has_imag:
                emit_y(yre, [(mtre, zre_h)])
                ntre = load_rows_on(ntre_d, "ntre", nc.sync)
                zim_h = [load_half(zim_d, "zim", 0, nc.sync),
                         load_half(zim_d, "zim", 1, nc.sync)]
                emit_u(ure, [(zre_h, ntre)])
                emit_y(yim, [(mtre, zim_h)])
                emit_u(uim, [(zim_h, ntre)])
            else:
                zim_h = [load_half(zim_d, "zim", 0, nc.sync),
                         load_half(zim_d, "zim", 1, nc.sync)]
                mtim = load_rows_on(mtim_d, "mtim", nc.sync)
                mtimn = load_rows_on(mtimn_d, "mtimn", nc.sync)
                ntre = load_rows_on(ntre_d, "ntre", nc.sync)
                ntim = load_rows_on(ntim_d, "ntim", nc.sync)
                ntimn = load_rows_on(ntimn_d, "ntimn", nc.sync)
                emit_y(yre, [(mtre, zre_h), (mtimn, zim_h)])
                emit_y(yim, [(mtre, zim_h), (mtim, zre_h)])
                emit_u(ure, [(zre_h, ntre), (zim_h, ntimn)])
                emit_u(uim, [(zim_h, ntre), (zre_h, ntim)])

            # -- P^T blocks (u-tile j, t-chunk n); zero upper regions -------
            pt = {}
            for j in range(TT):
                for n in range(2):
                    if n == 0 and j >= 4:
                        continue
                    ptile = cp.tile([P, 512], f32r, tag=f"pt{j}_{n}",
                                    name=f"pt{j}_{n}")
                    pt[(j, n)] = ptile
                    # the OUT clamp reads from col 256 even when the first
                    # transposed block starts later -> zero-fill the gap
                    lo = j * P - n * 512
                    if lo > 256:
                        nc.sync.dma_start(out=ptile[:, 256:lo],
                                          in_=zpad_d[:, 0:lo - 256])

            def emit_out_chunk(n, half=None, cols=(0, 512)):
                """out[:, n*512+cols] = U^T @ P^T for re and/or im."""
                c0, c1 = cols
                width = c1 - c0
                jmax = 4 * n + 3
                tsl = slice(n * 512 + c0, n * 512 + c1)
                pairs = ((ure, outre_d, nc.sync), (uim, outim_d, nc.sync))
                if half is not None:
                    pairs = (pairs[half],)
                js = [j for j in range(jmax + 1)
                      if max(c0, j * P - n * 512) < c1]
                for u, dram, oeng in pairs:
                    for m in range(CT):
                        msl = slice(m * P, (m + 1) * P)
                        ps = pmm.tile([P, 512], f32, tag="mm", name="psmm")
                        for j in js:
                            # pt[(j, n)] is all-zero left of column lo;
                            # clamp: N<256 f32r runs at 4 cyc/row
                            lo = min(max(c0, j * P - n * 512), c1 - 256)
                            lo = max(lo, c0)
                            _mm(nc, ps[:, lo - c0: width],
                                u[j][:, msl], pt[(j, n)][:, lo:c1],
                                start=(j == js[0]), stop=(j == js[-1]))
                        o = wp.tile([P, 512], f32, tag="osb", name="osb")
                        psum_to_sbuf(o[:, :width], ps[:, :width])
                        oeng.dma_start(out=dram[msl, tsl], in_=o[:, :width])

            # -- scores / softmax / transposes per t-tile -------------------
            def emit_scores_tile(i):
                ui = (i + 1) * P
                isl = slice((i % 4) * P, (i % 4 + 1) * P)
                s_sb = wp.tile([P, T], f32r, tag="s", name="s_sb")
                nchunks = (ui + 511) // 512
                lparts = []
                for q in range(nchunks):
                    w = min(512, ui - q * 512)
                    # widen 128-col chunks to 256: N<256 f32r matmuls run
                    # at 4 cyc/row, so the padded 256-col matmul is cheaper.
                    # Padded cols are masked to -inf -> exp 0.
                    wpad = max(w, 256) if q == nchunks - 1 else w
                    ps = pmm.tile([P, 512], f32, tag="mm", name="psmm")
                    k = 0
                    for zh, y in ((zre_h, yre), (zim_h, yim)):
                        for c in range(CT):
                            _mm(nc, ps[:, :wpad], zh[i // 4][c][:, isl],
                                y[c][q][:, :wpad],
                                start=(k == 0), stop=(k == 2 * CT - 1))
                            k += 1
                    last = q == nchunks - 1
                    if last:
                        fw = wpad - w + P   # frontier+pad width (128 or 256)
                        mask = tri if fw == P else trif
                        if wpad > fw:
                            # non-frontier part: exp straight from PSUM
                            lp = sp.tile([P, 1], f32, tag="lp", name="lp")
                            nc.scalar.activation(
                                out=s_sb[:, q * 512: q * 512 + wpad - fw],
                                in_=ps[:, : wpad - fw],
                                func=mybir.ActivationFunctionType.Exp,
                                accum_out=lp,
                            )
                            lparts.append(lp)
                        # frontier (+pad) cols: +mask (DVE), then exp
                        fr = sp.tile([P, 256], f32, tag="fr", name="fr")
                        nc.vector.tensor_add(out=fr[:, :fw],
                                             in0=ps[:, wpad - fw: wpad],
                                             in1=mask)
                        lp = sp.tile([P, 1], f32, tag="lp", name="lp")
                        nc.scalar.activation(
                            out=s_sb[:, ui - P: ui - P + fw],
                            in_=fr[:, :fw],
                            func=mybir.ActivationFunctionType.Exp,
                            accum_out=lp,
                        )
                        lparts.append(lp)
                    else:
                        lp = sp.tile([P, 1], f32, tag="lp", name="lp")
                        nc.scalar.activation(
                            out=s_sb[:, q * 512: q * 512 + w],
                            in_=ps[:, :w],
                            func=mybir.ActivationFunctionType.Exp,
                            accum_out=lp,
                        )
                        lparts.append(lp)

                lsum = lparts[0]
                for extra in lparts[1:]:
                    acc = sp.tile([P, 1], f32, tag="lacc", name="lacc")
                    nc.vector.tensor_add(out=acc, in0=lsum, in1=extra)
                    lsum = acc
                rl = sp.tile([P, 1], f32, tag="rl", name="rl")
                nc.vector.reciprocal(out=rl, in_=lsum)

                if DIAG_SCALE:
                    dg = sp.tile([P, P], f32r, tag="dg", name="dg")
                    nc.vector.tensor_scalar_mul(dg, ident, rl)
                    rhs = dg
                else:
                    nc.vector.tensor_scalar_mul(s_sb[:, :ui], s_sb[:, :ui],
                                                rl)
                    rhs = ident

                n = i // 4
                for j in range(i + 1):
                    pstile = ptr.tile([P, P], f32r, tag="tr", name="pstile")
                    nc.tensor.transpose(pstile, s_sb[:, j * P:(j + 1) * P],
                                        rhs)
                    nc.vector.tensor_copy(
                        out=pt[(j, n)][:, i * P - n * 512:
                                       (i + 1) * P - n * 512],
                        in_=pstile,
                    )

            for i in (4, 5, 6, 3):
                emit_scores_tile(i)
            emit_scores_tile(7)
            emit_scores_tile(2)
            emit_out_chunk(1, half=0)
            emit_scores_tile(1)
            emit_out_chunk(1, half=1)
            emit_scores_tile(0)
            emit_out_chunk(0, half=0)
            emit_out_chunk(0, half=1)

    nc.compile()
    return nc


def _prep_weights(Wq, phi_q, Wk, phi_k, Wv, phi_v, Wo, phi_o):
    Wq, Wk, Wv, Wo = (np.asarray(w, np.float64) for w in (Wq, Wk, Wv, Wo))
    pq, pk, pv, po = (np.asarray(p, np.float64)
                      for p in (phi_q, phi_k, phi_v, phi_o))
    M = (Wq.T @ (np.exp(1j * (pk - pq))[:, None] * Wk)) / math.sqrt(DH)
    N = (np.exp(1j * po)[:, None] * Wo) @ (np.exp(1j * pv)[:, None] * Wv)
    has_imag = not (np.allclose(M.imag, 0.0) and np.allclose(N.imag, 0.0))
    return M, N, has_imag


def kernel(z_re, z_im, Wq, phi_q, Wk, phi_k, Wv, phi_v, Wo, phi_o):
    z_re = np.ascontiguousarray(np.asarray(z_re, np.float32))
    z_im = np.ascontiguousarray(np.asarray(z_im, np.float32))
    M, N, has_imag = _prep_weights(Wq, phi_q, Wk, phi_k, Wv, phi_v, Wo, phi_o)

    mtre = np.ascontiguousarray(M.real.T.astype(np.float32))
    ntre = np.ascontiguousarray(N.real.T.astype(np.float32))
    consts = {"mtre": mtre, "ntre": ntre}
    if has_imag:
        mtim = np.ascontiguousarray(M.imag.T.astype(np.float32))
        ntim = np.ascontiguousarray(N.imag.T.astype(np.float32))
        consts.update(mtim=mtim, mtimn=-mtim, ntim=ntim, ntimn=-ntim)

    consts["ident"] = np.eye(P, dtype=np.float32)
    consts["tri"] = np.triu(np.full((P, P), NEG, np.float32), 1)
    consts["trif"] = np.concatenate(
        [np.triu(np.full((P, P), NEG, np.float32), 1),
         np.full((P, P), NEG, np.float32)], axis=1)
    consts["zpad"] = np.zeros((P, 384), np.float32)
    nc = _get_program(has_imag)
    in_maps = [
        dict(consts, zre=z_re[b].reshape(C, T), zim=z_im[b].reshape(C, T))
        for b in range(B)
    ]
    res = run_bass_kernel_spmd(nc, in_maps, list(range(B)))
    out_re = np.stack([res.results[b]["outre"].reshape(C, HH, WW)
                       for b in range(B)])
    out_im = np.stack([res.results[b]["outim"].reshape(C, HH, WW)
                       for b in range(B)])
    return out_re, out_im


# revision 20
# speedup vs baseline: 1.0980x; 1.0321x over previous
"""Trainium2 Bass kernel for nn_ComplexAttention (B=8, C=512, H=W=32, HEADS=8).

Strategy
--------
Data-parallel over batch: one batch element per NeuronCore (8 cores), no
collectives.  Host-side algebraic fusion shrinks the per-core work:

  reference:  Q = R_q Wq Z,  K = R_k Wk Z,  V = R_v Wv Z   (complex, [C,T])
              S = Re(Q^H K)/sqrt(dh),  causal softmax -> A
              out = R_o Wo (V A^T)

  fused:      M = Wq^T diag(e^{i(phi_k-phi_q)}) Wk / sqrt(dh)   (host, f64)
              N = diag(e^{i phi_o}) Wo diag(e^{i phi_v}) Wv     (host, f64)
              Y = M Z            (channel-major [C,T])
              S = Re(Z^H Y)      = Zre^T Yre + Zim^T Yim
              A = softmax(causal(S))        (no max-subtraction: |S| < ~30)
              U = N Z            (token-major [T,C])
              out = U^T A^T      (channel-major [C,T], = re/im pair)

Per-core tensor-engine work is ~320 [128x128x512] matmuls + 36 transposes.
Matmuls run as float32r (full-rate fp32 PE mode; PSUM accumulates fp32).

Schedule notes (from HW traces):
 - input DMA is BW-bound (~358 GB/s/core), so loads are interleaved with
   the first matmul phases (mtre+zre -> Y_re, ntre -> U_re, zim -> rest).
 - softmax exp reads scores straight out of PSUM (no copy), per-chunk
   partial row-sums are added on DVE afterwards.
 - the softmax 1/l is folded into the PE transpose: P^T blocks are
   computed as block.T @ diag(1/l) (diag built via ident * rl on DVE).
 - t-tiles 4..7 are processed first so the final out chunk (t 512..1023)
   overlaps the scores/softmax of t-tiles 0..3.
"""

import math

import numpy as np

import concourse.mybir as mybir
import concourse.tile as tile
from concourse import bacc
from concourse.bass_utils import run_bass_kernel_spmd

B, C, HH, WW = 8, 512, 32, 32
T = HH * WW          # 1024 tokens
DH = C // 8          # head dim (scale only)
P = 128
CT = C // P          # 4 channel tiles
TT = T // P          # 8 token tiles
NEG = -1.0e30
DIAG_SCALE = False   # PE transpose mode requires a permutation matrix

f32 = mybir.dt.float32
f32r = mybir.dt.float32r
bf16 = mybir.dt.bfloat16
VALUE_BF16 = True    # U / P / P^T path in bf16 (scores path stays f32r)


def _mm(nc, out, lhsT, rhs, start, stop):
    """matmul on float32r operands (1 cyc/row at N>=256)."""
    nc.tensor.matmul(out, lhsT, rhs, start=start, stop=stop)


_CACHE: dict = {}


def _get_program(has_imag: bool):
    key = has_imag
    if key not in _CACHE:
        _CACHE[key] = _build_program(has_imag)
    return _CACHE[key]


def _build_program(has_imag: bool):
    nc = bacc.Bacc("TRN2", target_bir_lowering=False, debug=False)

    zre_d = nc.dram_tensor("zre", [C, T], f32r, kind="ExternalInput").ap()
    zim_d = nc.dram_tensor("zim", [C, T], f32r, kind="ExternalInput").ap()
    mtre_d = nc.dram_tensor("mtre", [C, C], f32r, kind="ExternalInput").ap()
    ntre_d = nc.dram_tensor("ntre", [C, C], f32r, kind="ExternalInput").ap()
    if has_imag:
        mtim_d = nc.dram_tensor("mtim", [C, C], f32r, kind="ExternalInput").ap()
        mtimn_d = nc.dram_tensor("mtimn", [C, C], f32r, kind="ExternalInput").ap()
        ntim_d = nc.dram_tensor("ntim", [C, C], f32r, kind="ExternalInput").ap()
        ntimn_d = nc.dram_tensor("ntimn", [C, C], f32r, kind="ExternalInput").ap()
    vdt = bf16 if VALUE_BF16 else f32r
    ident_d = nc.dram_tensor("ident", [P, P], vdt, kind="ExternalInput").ap()
    tri_d = nc.dram_tensor("tri", [P, P], f32, kind="ExternalInput").ap()
    trif_d = nc.dram_tensor("trif", [P, 256], f32, kind="ExternalInput").ap()
    zpad_d = nc.dram_tensor("zpad", [P, 384], vdt, kind="ExternalInput").ap()
    outre_d = nc.dram_tensor("outre", [C, T], f32, kind="ExternalOutput").ap()
    outim_d = nc.dram_tensor("outim", [C, T], f32, kind="ExternalOutput").ap()

    with tile.TileContext(nc) as tc:
        with (
            tc.tile_pool(name="const", bufs=1) as cp,
            tc.tile_pool(name="work", bufs=3) as wp,
            tc.tile_pool(name="small", bufs=8) as sp,
            tc.tile_pool(name="psmm", bufs=4, space="PSUM") as pmm,
            tc.tile_pool(name="pstr", bufs=4, space="PSUM") as ptr,
        ):
            def load_rows_on(dram, tag, eng):
                tiles = []
                for c in range(CT):
                    t = cp.tile([P, C], f32r, tag=f"{tag}{c}",
                                name=f"{tag}{c}")
                    eng.dma_start(out=t, in_=dram[c * P:(c + 1) * P, :])
                    tiles.append(t)
                return tiles

            def load_half(dram, tag, half, eng):
                tiles = []
                for c in range(CT):
                    t = cp.tile([P, 512], f32r, tag=f"{tag}{c}_{half}",
                                name=f"{tag}{c}_{half}")
                    eng.dma_start(
                        out=t,
                        in_=dram[c * P:(c + 1) * P,
                                 half * 512:(half + 1) * 512])
                    tiles.append(t)
                return tiles

            # -- small constants + first compute inputs ---------------------
            # single sync queue (HBM BW is shared; parallel queues starve
            # the critical first loads), ordered by first use, with mtre/zre
            # interleaved per c-tile so accumulation starts after ~0.5MB.
            ident = cp.tile([P, P], vdt, tag="ident", name="ident")
            nc.sync.dma_start(out=ident, in_=ident_d)
            tri = cp.tile([P, P], f32, tag="tri", name="tri")
            nc.sync.dma_start(out=tri, in_=tri_d)
            trif = cp.tile([P, 256], f32, tag="trif", name="trif")
            nc.sync.dma_start(out=trif, in_=trif_d)
            mtre = [cp.tile([P, C], f32r, tag=f"mtre{c}", name=f"mtre{c}")
                    for c in range(CT)]
            zre_h = [[cp.tile([P, 512], f32r, tag=f"zre{c}_{h}",
                              name=f"zre{c}_{h}") for c in range(CT)]
                     for h in range(2)]
            for c in range(CT):
                nc.sync.dma_start(out=mtre[c],
                                  in_=mtre_d[c * P:(c + 1) * P, :])
                nc.sync.dma_start(out=zre_h[0][c],
                                  in_=zre_d[c * P:(c + 1) * P, 0:512])
            for c in range(CT):
                nc.sync.dma_start(out=zre_h[1][c],
                                  in_=zre_d[c * P:(c + 1) * P, 512:T])

            # persistent result tiles (split by column half: precise deps)
            yre = [[cp.tile([P, 512], f32r, tag=f"yre{c}_{n}",
                            name=f"yre{c}_{n}") for n in range(2)]
                   for c in range(CT)]
            yim = [[cp.tile([P, 512], f32r, tag=f"yim{c}_{n}",
                            name=f"yim{c}_{n}") for n in range(2)]
                   for c in range(CT)]
            ure = [cp.tile([P, C], vdt, tag=f"ure{j}", name=f"ure{j}")
                   for j in range(TT)]
            uim = [cp.tile([P, C], vdt, tag=f"uim{j}", name=f"uim{j}")
                   for j in range(TT)]

            def psum_to_sbuf(dst_ap, src_ap):
                nc.vector.tensor_copy(out=dst_ap, in_=src_ap)

            def emit_y(dst, terms):
                nterm = len(terms)
                for n in range(2):
                    pss = [pmm.tile([P, 512], f32, tag="mm", name="psmm")
                           for _ in range(CT)]
                    for t_i, (w, zh) in enumerate(terms):
                        for c in range(CT):
                            for m in range(CT):
                                _mm(nc, pss[m], w[c][:, m * P:(m + 1) * P],
                                    zh[n][c],
                                    start=(t_i == 0 and c == 0),
                                    stop=(t_i == nterm - 1 and c == CT - 1))
                    for m in range(CT):
                        psum_to_sbuf(dst[m][n], pss[m])

            def emit_u(dst, terms):
                for j in range(TT):
                    usl = slice((j % 4) * P, (j % 4 + 1) * P)
                    ps = pmm.tile([P, 512], f32, tag="mm", name="psmm")
                    nacc = len(terms) * CT
                    k = 0
                    for zh, w in terms:
                        for c in range(CT):
                            _mm(nc, ps, zh[j // 4][c][:, usl], w[c][:, :],
                                start=(k == 0), stop=(k == nacc - 1))
                            k += 1
                    psum_to_sbuf(dst[j], ps)

            # -- Y_re (needs mtre+zre only), then stream in the rest.
            # Later loads are EMITTED after emit_y so the watermark-style
            # sem waits on the first matmuls don't cover them; the DMA
            # engines still run their own streams immediately.
            if not has_imag:
                emit_y(yre, [(mtre, zre_h)])
                ntre = load_rows_on(ntre_d, "ntre", nc.sync)
                zim_h = [load_half(zim_d, "zim", 0, nc.sync),
                         load_half(zim_d, "zim", 1, nc.sync)]
                emit_u(ure, [(zre_h, ntre)])
                emit_y(yim, [(mtre, zim_h)])
                emit_u(uim, [(zim_h, ntre)])
            else:
                zim_h = [load_half(zim_d, "zim", 0, nc.sync),
                         load_half(zim_d, "zim", 1, nc.sync)]
                mtim = load_rows_on(mtim_d, "mtim", nc.sync)
                mtimn = load_rows_on(mtimn_d, "mtimn", nc.sync)
                ntre = load_rows_on(ntre_d, "ntre", nc.sync)
                ntim = load_rows_on(ntim_d, "ntim", nc.sync)
                ntimn = load_rows_on(ntimn_d, "ntimn", nc.sync)
                emit_y(yre, [(mtre, zre_h), (mtimn, zim_h)])
                emit_y(yim, [(mtre, zim_h), (mtim, zre_h)])
                emit_u(ure, [(zre_h, ntre), (zim_h, ntimn)])
                emit_u(uim, [(zim_h, ntre), (zre_h, ntim)])

            # -- P^T blocks (u-tile j, t-chunk n); zero upper regions -------
            pt = {}
            for j in range(TT):
                for n in range(2):
                    if n == 0 and j >= 4:
                        continue
                    ptile = cp.tile([P, 512], vdt, tag=f"pt{j}_{n}",
                                    name=f"pt{j}_{n}")
                    pt[(j, n)] = ptile
                    # the OUT clamp reads from col 256 even when the first
                    # transposed block starts later -> zero-fill the gap
                    lo = j * P - n * 512
                    if lo > 256:
                        nc.sync.dma_start(out=ptile[:, 256:lo],
                                          in_=zpad_d[:, 0:lo - 256])

            def emit_out_chunk(n, half=None, cols=(0, 512)):
                """out[:, n*512+cols] = U^T @ P^T for re and/or im."""
                c0, c1 = cols
                width = c1 - c0
                jmax = 4 * n + 3
                tsl = slice(n * 512 + c0, n * 512 + c1)
                pairs = ((ure, outre_d, nc.sync), (uim, outim_d, nc.sync))
                if half is not None:
                    pairs = (pairs[half],)
                js = [j for j in range(jmax + 1)
                      if max(c0, j * P - n * 512) < c1]
                for u, dram, oeng in pairs:
                    for m in range(CT):
                        msl = slice(m * P, (m + 1) * P)
                        ps = pmm.tile([P, 512], f32, tag="mm", name="psmm")
                        for j in js:
                            # pt[(j, n)] is all-zero left of column lo;
                            # clamp: N<256 f32r runs at 4 cyc/row
                            lo = min(max(c0, j * P - n * 512), c1 - 256)
                            lo = max(lo, c0)
                            _mm(nc, ps[:, lo - c0: width],
                                u[j][:, msl], pt[(j, n)][:, lo:c1],
                                start=(j == js[0]), stop=(j == js[-1]))
                        o = wp.tile([P, 512], f32, tag="osb", name="osb")
                        psum_to_sbuf(o[:, :width], ps[:, :width])
                        oeng.dma_start(out=dram[msl, tsl], in_=o[:, :width])

            # -- scores / softmax / transposes per t-tile -------------------
            def emit_scores_tile(i):
                ui = (i + 1) * P
                isl = slice((i % 4) * P, (i % 4 + 1) * P)
                s_sb = wp.tile([P, T], vdt, tag="s", name="s_sb")
                nchunks = (ui + 511) // 512
                lparts = []
                for q in range(nchunks):
                    w = min(512, ui - q * 512)
                    # widen 128-col chunks to 256: N<256 f32r matmuls run
                    # at 4 cyc/row, so the padded 256-col matmul is cheaper.
                    # Padded cols are masked to -inf -> exp 0.
                    wpad = max(w, 256) if q == nchunks - 1 else w
                    ps = pmm.tile([P, 512], f32, tag="mm", name="psmm")
                    k = 0
                    for zh, y in ((zre_h, yre), (zim_h, yim)):
                        for c in range(CT):
                            _mm(nc, ps[:, :wpad], zh[i // 4][c][:, isl],
                                y[c][q][:, :wpad],
                                start=(k == 0), stop=(k == 2 * CT - 1))
                            k += 1
                    last = q == nchunks - 1
                    if last:
                        fw = wpad - w + P   # frontier+pad width (128 or 256)
                        mask = tri if fw == P else trif
                        if wpad > fw:
                            # non-frontier part: exp straight from PSUM
                            lp = sp.tile([P, 1], f32, tag="lp", name="lp")
                            nc.scalar.activation(
                                out=s_sb[:, q * 512: q * 512 + wpad - fw],
                                in_=ps[:, : wpad - fw],
                                func=mybir.ActivationFunctionType.Exp,
                                accum_out=lp,
                            )
                            lparts.append(lp)
                        # frontier (+pad) cols: +mask (DVE), then exp
                        fr = sp.tile([P, 256], f32, tag="fr", name="fr")
                        nc.vector.tensor_add(out=fr[:, :fw],
                                             in0=ps[:, wpad - fw: wpad],
                                             in1=mask)
                        lp = sp.tile([P, 1], f32, tag="lp", name="lp")
                        nc.scalar.activation(
                            out=s_sb[:, ui - P: ui - P + fw],
                            in_=fr[:, :fw],
                            func=mybir.ActivationFunctionType.Exp,
                            accum_out=lp,
                        )
                        lparts.append(lp)
                    else:
                        lp = sp.tile([P, 1], f32, tag="lp", name="lp")
                        nc.scalar.activation(
                            out=s_sb[:, q * 512: q * 512 + w],
                            in_=ps[:, :w],
                            func=mybir.ActivationFunctionType.Exp,
                            accum_out=lp,
                        )
                        lparts.append(lp)

                lsum = lparts[0]
                for extra in lparts[1:]:
                    acc = sp.tile([P, 1], f32, tag="lacc", name="lacc")
                    nc.vector.tensor_add(out=acc, in0=lsum, in1=extra)
                    lsum = acc
                rl = sp.tile([P, 1], f32, tag="rl", name="rl")
                nc.vector.reciprocal(out=rl, in_=lsum)

                if DIAG_SCALE:
                    dg = sp.tile([P, P], f32r, tag="dg", name="dg")
                    nc.vector.tensor_scalar_mul(dg, ident, rl)
                    rhs = dg
                else:
                    nc.vector.tensor_scalar_mul(s_sb[:, :ui], s_sb[:, :ui],
                                                rl)
                    rhs = ident

                n = i // 4
                for j in range(i + 1):
                    pstile = ptr.tile([P, P], vdt, tag="tr", name="pstile")
                    nc.tensor.transpose(pstile, s_sb[:, j * P:(j + 1) * P],
                                        rhs)
                    nc.vector.tensor_copy(
                        out=pt[(j, n)][:, i * P - n * 512:
                                       (i + 1) * P - n * 512],
                        in_=pstile,
                    )

            for i in (4, 5, 6, 3):
                emit_scores_tile(i)
            emit_scores_tile(7)
            emit_scores_tile(2)
            emit_out_chunk(1, half=0)
            emit_scores_tile(1)
            emit_out_chunk(1, half=1)
            emit_scores_tile(0)
            emit_out_chunk(0, half=0)
            emit_out_chunk(0, half=1)

    nc.compile()
    return nc


def _prep_weights(Wq, phi_q, Wk, phi_k, Wv, phi_v, Wo, phi_o):
    Wq, Wk, Wv, Wo = (np.asarray(w, np.float64) for w in (Wq, Wk, Wv, Wo))
    pq, pk, pv, po = (np.asarray(p, np.float64)
                      for p in (phi_q, phi_k, phi_v, phi_o))
    M = (Wq.T @ (np.exp(1j * (pk - pq))[:, None] * Wk)) / math.sqrt(DH)
    N = (np.exp(1j * po)[:, None] * Wo) @ (np.exp(1j * pv)[:, None] * Wv)
    has_imag = not (np.allclose(M.imag, 0.0) and np.allclose(N.imag, 0.0))
    return M, N, has_imag


def kernel(z_re, z_im, Wq, phi_q, Wk, phi_k, Wv, phi_v, Wo, phi_o):
    z_re = np.ascontiguousarray(np.asarray(z_re, np.float32))
    z_im = np.ascontiguousarray(np.asarray(z_im, np.float32))
    M, N, has_imag = _prep_weights(Wq, phi_q, Wk, phi_k, Wv, phi_v, Wo, phi_o)

    mtre = np.ascontiguousarray(M.real.T.astype(np.float32))
    ntre = np.ascontiguousarray(N.real.T.astype(np.float32))
    consts = {"mtre": mtre, "ntre": ntre}
    if has_imag:
        mtim = np.ascontiguousarray(M.imag.T.astype(np.float32))
        ntim = np.ascontiguousarray(N.imag.T.astype(np.float32))
        consts.update(mtim=mtim, mtimn=-mtim, ntim=ntim, ntimn=-ntim)

    import ml_dtypes
    vnp = ml_dtypes.bfloat16 if VALUE_BF16 else np.float32
    consts["ident"] = np.eye(P, dtype=vnp)
    consts["tri"] = np.triu(np.full((P, P), NEG, np.float32), 1)
    consts["trif"] = np.concatenate(
        [np.triu(np.full((P, P), NEG, np.float32), 1),
         np.full((P, P), NEG, np.float32)], axis=1)
    consts["zpad"] = np.zeros((P, 384), vnp)
    nc = _get_program(has_imag)
    in_maps = [
        dict(consts, zre=z_re[b].reshape(C, T), zim=z_im[b].reshape(C, T))
        for b in range(B)
    ]
    res = run_bass_kernel_spmd(nc, in_maps, list(range(B)))
    out_re = np.stack([res.results[b]["outre"].reshape(C, HH, WW)
                       for b in range(B)])
    out_im = np.stack([res.results[b]["outim"].reshape(C, HH, WW)
                       for b in range(B)])
    return out_re, out_im


# revision 21
# speedup vs baseline: 1.1106x; 1.0114x over previous
"""Trainium2 Bass kernel for nn_ComplexAttention (B=8, C=512, H=W=32, HEADS=8).

Strategy
--------
Data-parallel over batch: one batch element per NeuronCore (8 cores), no
collectives.  Host-side algebraic fusion shrinks the per-core work:

  reference:  Q = R_q Wq Z,  K = R_k Wk Z,  V = R_v Wv Z   (complex, [C,T])
              S = Re(Q^H K)/sqrt(dh),  causal softmax -> A
              out = R_o Wo (V A^T)

  fused:      M = Wq^T diag(e^{i(phi_k-phi_q)}) Wk / sqrt(dh)   (host, f64)
              N = diag(e^{i phi_o}) Wo diag(e^{i phi_v}) Wv     (host, f64)
              Y = M Z            (channel-major [C,T])
              S = Re(Z^H Y)      = Zre^T Yre + Zim^T Yim
              A = softmax(causal(S))        (no max-subtraction: |S| < ~30)
              U = N Z            (token-major [T,C])
              out = U^T A^T      (channel-major [C,T], = re/im pair)

Per-core tensor-engine work is ~320 [128x128x512] matmuls + 36 transposes.
Matmuls run as float32r (full-rate fp32 PE mode; PSUM accumulates fp32).

Schedule notes (from HW traces):
 - input DMA is BW-bound (~358 GB/s/core), so loads are interleaved with
   the first matmul phases (mtre+zre -> Y_re, ntre -> U_re, zim -> rest).
 - softmax exp reads scores straight out of PSUM (no copy), per-chunk
   partial row-sums are added on DVE afterwards.
 - the softmax 1/l is folded into the PE transpose: P^T blocks are
   computed as block.T @ diag(1/l) (diag built via ident * rl on DVE).
 - t-tiles 4..7 are processed first so the final out chunk (t 512..1023)
   overlaps the scores/softmax of t-tiles 0..3.
"""

import math

import numpy as np

import concourse.mybir as mybir
import concourse.tile as tile
from concourse import bacc
from concourse.bass_utils import run_bass_kernel_spmd

B, C, HH, WW = 8, 512, 32, 32
T = HH * WW          # 1024 tokens
DH = C // 8          # head dim (scale only)
P = 128
CT = C // P          # 4 channel tiles
TT = T // P          # 8 token tiles
NEG = -1.0e30
DIAG_SCALE = False   # PE transpose mode requires a permutation matrix

f32 = mybir.dt.float32
f32r = mybir.dt.float32r
bf16 = mybir.dt.bfloat16
VALUE_BF16 = True    # U / P / P^T path in bf16 (scores path stays f32r)


def _mm(nc, out, lhsT, rhs, start, stop):
    """matmul on float32r operands (1 cyc/row at N>=256)."""
    nc.tensor.matmul(out, lhsT, rhs, start=start, stop=stop)


_CACHE: dict = {}


def _get_program(has_imag: bool):
    key = has_imag
    if key not in _CACHE:
        _CACHE[key] = _build_program(has_imag)
    return _CACHE[key]


def _build_program(has_imag: bool):
    nc = bacc.Bacc("TRN2", target_bir_lowering=False, debug=False)

    zre_d = nc.dram_tensor("zre", [C, T], f32r, kind="ExternalInput").ap()
    zim_d = nc.dram_tensor("zim", [C, T], f32r, kind="ExternalInput").ap()
    mtre_d = nc.dram_tensor("mtre", [C, C], f32r, kind="ExternalInput").ap()
    ntre_d = nc.dram_tensor("ntre", [C, C], f32r, kind="ExternalInput").ap()
    if has_imag:
        mtim_d = nc.dram_tensor("mtim", [C, C], f32r, kind="ExternalInput").ap()
        mtimn_d = nc.dram_tensor("mtimn", [C, C], f32r, kind="ExternalInput").ap()
        ntim_d = nc.dram_tensor("ntim", [C, C], f32r, kind="ExternalInput").ap()
        ntimn_d = nc.dram_tensor("ntimn", [C, C], f32r, kind="ExternalInput").ap()
    vdt = bf16 if VALUE_BF16 else f32r
    ident_d = nc.dram_tensor("ident", [P, P], vdt, kind="ExternalInput").ap()
    tri_d = nc.dram_tensor("tri", [P, P], f32, kind="ExternalInput").ap()
    trif_d = nc.dram_tensor("trif", [P, 256], f32, kind="ExternalInput").ap()
    zpad_d = nc.dram_tensor("zpad", [P, 384], vdt, kind="ExternalInput").ap()
    outre_d = nc.dram_tensor("outre", [C, T], f32, kind="ExternalOutput").ap()
    outim_d = nc.dram_tensor("outim", [C, T], f32, kind="ExternalOutput").ap()

    with tile.TileContext(nc) as tc:
        with (
            tc.tile_pool(name="const", bufs=1) as cp,
            tc.tile_pool(name="work", bufs=3) as wp,
            tc.tile_pool(name="small", bufs=8) as sp,
            tc.tile_pool(name="psmm", bufs=4, space="PSUM") as pmm,
            tc.tile_pool(name="pstr", bufs=4, space="PSUM") as ptr,
        ):
            def load_rows_on(dram, tag, eng):
                tiles = []
                for c in range(CT):
                    t = cp.tile([P, C], f32r, tag=f"{tag}{c}",
                                name=f"{tag}{c}")
                    eng.dma_start(out=t, in_=dram[c * P:(c + 1) * P, :])
                    tiles.append(t)
                return tiles

            def load_half(dram, tag, half, eng):
                tiles = []
                for c in range(CT):
                    t = cp.tile([P, 512], f32r, tag=f"{tag}{c}_{half}",
                                name=f"{tag}{c}_{half}")
                    eng.dma_start(
                        out=t,
                        in_=dram[c * P:(c + 1) * P,
                                 half * 512:(half + 1) * 512])
                    tiles.append(t)
                return tiles

            # -- small constants + first compute inputs ---------------------
            # single sync queue (HBM BW is shared; parallel queues starve
            # the critical first loads), ordered by first use, with mtre/zre
            # interleaved per c-tile so accumulation starts after ~0.5MB.
            ident = cp.tile([P, P], vdt, tag="ident", name="ident")
            nc.sync.dma_start(out=ident, in_=ident_d)
            tri = cp.tile([P, P], f32, tag="tri", name="tri")
            nc.sync.dma_start(out=tri, in_=tri_d)
            trif = cp.tile([P, 256], f32, tag="trif", name="trif")
            nc.sync.dma_start(out=trif, in_=trif_d)
            mtre = [cp.tile([P, C], f32r, tag=f"mtre{c}", name=f"mtre{c}")
                    for c in range(CT)]
            zre_h = [[cp.tile([P, 512], f32r, tag=f"zre{c}_{h}",
                              name=f"zre{c}_{h}") for c in range(CT)]
                     for h in range(2)]
            for c in range(CT):
                nc.sync.dma_start(out=mtre[c],
                                  in_=mtre_d[c * P:(c + 1) * P, :])
                nc.sync.dma_start(out=zre_h[0][c],
                                  in_=zre_d[c * P:(c + 1) * P, 0:512])
            for c in range(CT):
                nc.sync.dma_start(out=zre_h[1][c],
                                  in_=zre_d[c * P:(c + 1) * P, 512:T])

            # persistent result tiles (split by column half: precise deps)
            yre = [[cp.tile([P, 512], f32r, tag=f"yre{c}_{n}",
                            name=f"yre{c}_{n}") for n in range(2)]
                   for c in range(CT)]
            yim = [[cp.tile([P, 512], f32r, tag=f"yim{c}_{n}",
                            name=f"yim{c}_{n}") for n in range(2)]
                   for c in range(CT)]
            ure = [cp.tile([P, C], vdt, tag=f"ure{j}", name=f"ure{j}")
                   for j in range(TT)]
            uim = [cp.tile([P, C], vdt, tag=f"uim{j}", name=f"uim{j}")
                   for j in range(TT)]

            def psum_to_sbuf(dst_ap, src_ap):
                nc.vector.tensor_copy(out=dst_ap, in_=src_ap)

            def emit_y(dst, terms):
                nterm = len(terms)
                for n in range(2):
                    pss = [pmm.tile([P, 512], f32, tag="mm", name="psmm")
                           for _ in range(CT)]
                    for t_i, (w, zh) in enumerate(terms):
                        for c in range(CT):
                            for m in range(CT):
                                _mm(nc, pss[m], w[c][:, m * P:(m + 1) * P],
                                    zh[n][c],
                                    start=(t_i == 0 and c == 0),
                                    stop=(t_i == nterm - 1 and c == CT - 1))
                    for m in range(CT):
                        psum_to_sbuf(dst[m][n], pss[m])

            def emit_u(dst, terms):
                for j in range(TT):
                    usl = slice((j % 4) * P, (j % 4 + 1) * P)
                    ps = pmm.tile([P, 512], f32, tag="mm", name="psmm")
                    nacc = len(terms) * CT
                    k = 0
                    for zh, w in terms:
                        for c in range(CT):
                            _mm(nc, ps, zh[j // 4][c][:, usl], w[c][:, :],
                                start=(k == 0), stop=(k == nacc - 1))
                            k += 1
                    psum_to_sbuf(dst[j], ps)

            # -- Y_re (needs mtre+zre only), then stream in the rest.
            # Later loads are EMITTED after emit_y so the watermark-style
            # sem waits on the first matmuls don't cover them; the DMA
            # engines still run their own streams immediately.
            if not has_imag:
                emit_y(yre, [(mtre, zre_h)])
                ntre = load_rows_on(ntre_d, "ntre", nc.sync)
                zim_h = [load_half(zim_d, "zim", 0, nc.sync),
                         load_half(zim_d, "zim", 1, nc.sync)]
                emit_u(ure, [(zre_h, ntre)])
                emit_y(yim, [(mtre, zim_h)])
                emit_u(uim, [(zim_h, ntre)])
            else:
                zim_h = [load_half(zim_d, "zim", 0, nc.sync),
                         load_half(zim_d, "zim", 1, nc.sync)]
                mtim = load_rows_on(mtim_d, "mtim", nc.sync)
                mtimn = load_rows_on(mtimn_d, "mtimn", nc.sync)
                ntre = load_rows_on(ntre_d, "ntre", nc.sync)
                ntim = load_rows_on(ntim_d, "ntim", nc.sync)
                ntimn = load_rows_on(ntimn_d, "ntimn", nc.sync)
                emit_y(yre, [(mtre, zre_h), (mtimn, zim_h)])
                emit_y(yim, [(mtre, zim_h), (mtim, zre_h)])
                emit_u(ure, [(zre_h, ntre), (zim_h, ntimn)])
                emit_u(uim, [(zim_h, ntre), (zre_h, ntim)])

            # -- P^T blocks (u-tile j, t-chunk n); zero upper regions -------
            pt = {}
            for j in range(TT):
                for n in range(2):
                    if n == 0 and j >= 4:
                        continue
                    ptile = cp.tile([P, 512], vdt, tag=f"pt{j}_{n}",
                                    name=f"pt{j}_{n}")
                    pt[(j, n)] = ptile
                    # the OUT clamp reads from col 256 even when the first
                    # transposed block starts later -> zero-fill the gap
                    lo = j * P - n * 512
                    if lo > 256:
                        nc.sync.dma_start(out=ptile[:, 256:lo],
                                          in_=zpad_d[:, 0:lo - 256])

            def emit_out_chunk(n, half=None, cols=(0, 512)):
                """out[:, n*512+cols] = U^T @ P^T for re and/or im."""
                c0, c1 = cols
                width = c1 - c0
                jmax = 4 * n + 3
                tsl = slice(n * 512 + c0, n * 512 + c1)
                pairs = ((ure, outre_d, nc.sync), (uim, outim_d, nc.sync))
                if half is not None:
                    pairs = (pairs[half],)
                js = [j for j in range(jmax + 1)
                      if max(c0, j * P - n * 512) < c1]
                for u, dram, oeng in pairs:
                    # one combined SBUF tile -> single 1MB DMA per half
                    o = wp.tile([P, CT, 512], f32, tag="osb", name="osb")
                    for m in range(CT):
                        msl = slice(m * P, (m + 1) * P)
                        ps = pmm.tile([P, 512], f32, tag="mm", name="psmm")
                        for j in js:
                            # pt[(j, n)] is all-zero left of column lo;
                            # clamp: N<256 f32r runs at 4 cyc/row
                            lo = min(max(c0, j * P - n * 512), c1 - 256)
                            lo = max(lo, c0)
                            _mm(nc, ps[:, lo - c0: width],
                                u[j][:, msl], pt[(j, n)][:, lo:c1],
                                start=(j == js[0]), stop=(j == js[-1]))
                        psum_to_sbuf(o[:, m, :width], ps[:, :width])
                    dview = dram.rearrange("(m p) t -> p m t", p=P)
                    oeng.dma_start(out=dview[:, :, tsl], in_=o[:, :, :width])

            # -- scores / softmax / transposes per t-tile -------------------
            def emit_scores_tile(i):
                ui = (i + 1) * P
                isl = slice((i % 4) * P, (i % 4 + 1) * P)
                s_sb = wp.tile([P, T], vdt, tag="s", name="s_sb")
                nchunks = (ui + 511) // 512
                lparts = []
                for q in range(nchunks):
                    w = min(512, ui - q * 512)
                    # widen 128-col chunks to 256: N<256 f32r matmuls run
                    # at 4 cyc/row, so the padded 256-col matmul is cheaper.
                    # Padded cols are masked to -inf -> exp 0.
                    wpad = max(w, 256) if q == nchunks - 1 else w
                    ps = pmm.tile([P, 512], f32, tag="mm", name="psmm")
                    k = 0
                    for zh, y in ((zre_h, yre), (zim_h, yim)):
                        for c in range(CT):
                            _mm(nc, ps[:, :wpad], zh[i // 4][c][:, isl],
                                y[c][q][:, :wpad],
                                start=(k == 0), stop=(k == 2 * CT - 1))
                            k += 1
                    last = q == nchunks - 1
                    if last:
                        fw = wpad - w + P   # frontier+pad width (128 or 256)
                        mask = tri if fw == P else trif
                        if wpad > fw:
                            # non-frontier part: exp straight from PSUM
                            lp = sp.tile([P, 1], f32, tag="lp", name="lp")
                            nc.scalar.activation(
                                out=s_sb[:, q * 512: q * 512 + wpad - fw],
                                in_=ps[:, : wpad - fw],
                                func=mybir.ActivationFunctionType.Exp,
                                accum_out=lp,
                            )
                            lparts.append(lp)
                        # frontier (+pad) cols: +mask (DVE), then exp
                        fr = sp.tile([P, 256], f32, tag="fr", name="fr")
                        nc.vector.tensor_add(out=fr[:, :fw],
                                             in0=ps[:, wpad - fw: wpad],
                                             in1=mask)
                        lp = sp.tile([P, 1], f32, tag="lp", name="lp")
                        nc.scalar.activation(
                            out=s_sb[:, ui - P: ui - P + fw],
                            in_=fr[:, :fw],
                            func=mybir.ActivationFunctionType.Exp,
                            accum_out=lp,
                        )
                        lparts.append(lp)
                    else:
                        lp = sp.tile([P, 1], f32, tag="lp", name="lp")
                        nc.scalar.activation(
                            out=s_sb[:, q * 512: q * 512 + w],
                            in_=ps[:, :w],
                            func=mybir.ActivationFunctionType.Exp,
                            accum_out=lp,
                        )
                        lparts.append(lp)

                lsum = lparts[0]
                for extra in lparts[1:]:
                    acc = sp.tile([P, 1], f32, tag="lacc", name="lacc")
                    nc.vector.tensor_add(out=acc, in0=lsum, in1=extra)
                    lsum = acc
                rl = sp.tile([P, 1], f32, tag="rl", name="rl")
                nc.vector.reciprocal(out=rl, in_=lsum)

                if DIAG_SCALE:
                    dg = sp.tile([P, P], f32r, tag="dg", name="dg")
                    nc.vector.tensor_scalar_mul(dg, ident, rl)
                    rhs = dg
                else:
                    nc.vector.tensor_scalar_mul(s_sb[:, :ui], s_sb[:, :ui],
                                                rl)
                    rhs = ident

                n = i // 4
                for j in range(i + 1):
                    pstile = ptr.tile([P, P], vdt, tag="tr", name="pstile")
                    nc.tensor.transpose(pstile, s_sb[:, j * P:(j + 1) * P],
                                        rhs)
                    nc.vector.tensor_copy(
                        out=pt[(j, n)][:, i * P - n * 512:
                                       (i + 1) * P - n * 512],
                        in_=pstile,
                    )

            for i in (4, 5, 6, 3):
                emit_scores_tile(i)
            emit_scores_tile(7)
            emit_scores_tile(2)
            emit_out_chunk(1, half=0)
            emit_scores_tile(1)
            emit_out_chunk(1, half=1)
            emit_scores_tile(0)
            emit_out_chunk(0, half=0)
            emit_out_chunk(0, half=1)

    nc.compile()
    return nc


def _prep_weights(Wq, phi_q, Wk, phi_k, Wv, phi_v, Wo, phi_o):
    Wq, Wk, Wv, Wo = (np.asarray(w, np.float64) for w in (Wq, Wk, Wv, Wo))
    pq, pk, pv, po = (np.asarray(p, np.float64)
                      for p in (phi_q, phi_k, phi_v, phi_o))
    M = (Wq.T @ (np.exp(1j * (pk - pq))[:, None] * Wk)) / math.sqrt(DH)
    N = (np.exp(1j * po)[:, None] * Wo) @ (np.exp(1j * pv)[:, None] * Wv)
    has_imag = not (np.allclose(M.imag, 0.0) and np.allclose(N.imag, 0.0))
    return M, N, has_imag


def kernel(z_re, z_im, Wq, phi_q, Wk, phi_k, Wv, phi_v, Wo, phi_o):
    z_re = np.ascontiguousarray(np.asarray(z_re, np.float32))
    z_im = np.ascontiguousarray(np.asarray(z_im, np.float32))
    M, N, has_imag = _prep_weights(Wq, phi_q, Wk, phi_k, Wv, phi_v, Wo, phi_o)

    mtre = np.ascontiguousarray(M.real.T.astype(np.float32))
    ntre = np.ascontiguousarray(N.real.T.astype(np.float32))
    consts = {"mtre": mtre, "ntre": ntre}
    if has_imag:
        mtim = np.ascontiguousarray(M.imag.T.astype(np.float32))
        ntim = np.ascontiguousarray(N.imag.T.astype(np.float32))
        consts.update(mtim=mtim, mtimn=-mtim, ntim=ntim, ntimn=-ntim)

    import ml_dtypes
    vnp = ml_dtypes.bfloat16 if VALUE_BF16 else np.float32
    consts["ident"] = np.eye(P, dtype=vnp)
    consts["tri"] = np.triu(np.full((P, P), NEG, np.float32), 1)
    consts["trif"] = np.concatenate(
        [np.triu(np.full((P, P), NEG, np.float32), 1),
         np.full((P, P), NEG, np.float32)], axis=1)
    consts["zpad"] = np.zeros((P, 384), vnp)
    nc = _get_program(has_imag)
    in_maps = [
        dict(consts, zre=z_re[b].reshape(C, T), zim=z_im[b].reshape(C, T))
        for b in range(B)
    ]
    res = run_bass_kernel_spmd(nc, in_maps, list(range(B)))
    out_re = np.stack([res.results[b]["outre"].reshape(C, HH, WW)
                       for b in range(B)])
    out_im = np.stack([res.results[b]["outim"].reshape(C, HH, WW)
                       for b in range(B)])
    return out_re, out_im
